# revision 29
# baseline (speedup 1.0000x reference)
"""Trainium2 Bass kernel for banded local attention.

Reference computation (B=2, S=2048, D=512, H=8, dh=64, local_range=7):
  q = hs @ Wq, k = hs @ Wk (per-head slices)
  scores = q k^T / sqrt(dh); w = softmax(scores) * band; w /= sum(w) + 1e-6
  ctx = w @ hs                                  -> [B, H, S, D]

Since w is re-normalized over the band, softmax(scores)*band/sum ==
band-limited softmax up to the tiny 1e-6*Z correction (~1e-4 relative),
so we only ever compute the 15-diagonal band of scores.

Sharding: sequence-sharded. Core c handles batch c//4, rows
[512*(c%4), 512*(c%4)+512), ALL 8 heads. Each core loads only its
~526-row hs window (plus replicated Wq/Wk), so per-core input DMA is
~2.3 MB instead of the ~4.3 MB a head-sharded split would need.

Row tiling: R=114-row output tiles have a j-window of R+14 = 128 rows,
which exactly fits one 128-partition matmul contraction. Scores are
computed directly TRANSPOSED (psum[j, i] via lhsT=kT, rhs=qT), so E^T
is available for the context matmul without any PE transpose, and the
context is ONE 512-col matmul per (tile, head). Band softmax weights
are fully normalized in SBUF before the context matmul (mask-multiply,
partition_all_reduce rowsum, divide - all on the otherwise-idle Pool
engine), so context psums hold final values and evict as plain 2-head
copies split between ACT and DVE.

Output is written bf16 (halves the dominant output DMA) and upcast to
f32 on the host.
"""

import os
import numpy as np
import ml_dtypes

DBG = set(os.environ.get("K_DBG", "").split(","))

BF = ml_dtypes.bfloat16
S, D, H, DH = 2048, 512, 8, 64
NCORES = 8
CR = 512           # rows per core
NT = 5             # row tiles per core (4*114 + 56)
WIN = 526          # hs window rows (512 + 2*7)
WINPAD = 584       # padded window so tile slots are uniform (114*4+128)
R_LIST = [114, 114, 114, 114, 56]
W_LIST = [128, 128, 128, 128, 70]

TRACE = False
LAST_RESULTS = None

_NC_CACHE = {}


def _build_nc():
    import concourse.bacc as bacc
    import concourse.mybir as mybir
    import concourse.tile as tile
    from concourse import bass_isa

    f32 = mybir.dt.float32
    bf16 = mybir.dt.bfloat16
    AF = mybir.ActivationFunctionType
    MUL = mybir.AluOpType.mult
    DIV = mybir.AluOpType.divide

    nc = bacc.Bacc("TRN2", target_bir_lowering=False, debug=False, num_devices=NCORES)

    # lhsT-packed projections: [d%128, d//128, hd]; q cols 0:512, k 512:1024
    wqk = nc.dram_tensor("wqk", [128, 4, 1024], bf16, kind="ExternalInput").ap()
    # transposed hs window: hst[p, dc, s] = hs_pad[s, 128*dc+p]
    hst = nc.dram_tensor("hst", [128, 4, WIN], bf16, kind="ExternalInput").ap()
    # banded hs window slots: hsw[p, t, d] = hs_pad[114t+p, d]
    hsw = nc.dram_tensor("hsw", [128, 5, 512], bf16, kind="ExternalInput").ap()
    # multiplicative band mask in [j, i] layout, per tile
    maskt = nc.dram_tensor("maskt", [128, 5, 114], bf16, kind="ExternalInput").ap()
    out = nc.dram_tensor("out", [H, CR, D], bf16, kind="ExternalOutput").ap()
    out_r = out.rearrange("h s d -> s h d")

    with tile.TileContext(nc) as tc:
        with (
            tc.tile_pool(name="const", bufs=1) as cpool,
            tc.tile_pool(name="ework", bufs=4) as epool,
            tc.tile_pool(name="owork", bufs=2) as opool,
            tc.tile_pool(name="psc", bufs=2, space="PSUM") as pscp,
            tc.tile_pool(name="pbig", bufs=3, space="PSUM") as pbig,
        ):
            # ---- input loads (chunked so the first proj matmul starts early)
            wqk_sb = cpool.tile([128, 4, 1024], bf16)
            hst_sb = cpool.tile([128, 4, WIN], bf16)
            for dc in range(4):
                nc.sync.dma_start(out=wqk_sb[:, dc], in_=wqk[:, dc])
                nc.sync.dma_start(out=hst_sb[:, dc], in_=hst[:, dc])
            hsw_sb = cpool.tile([128, 5, 512], bf16)
            nc.sync.dma_start(out=hsw_sb, in_=hsw)
            mask_sb = cpool.tile([128, 5, 114], bf16)
            nc.sync.dma_start(out=mask_sb, in_=maskt)

            # ---- projections ----
            # The PE faults when 64-contraction matmuls at different tile
            # positions get scheduled back-to-back, so every matmul here
            # uses the full 128-partition contraction. The per-head scores
            # contraction (64 of the 128 hd lanes) is realized by storing q
            # zero-padded per head: qTz plane `head` holds q values on
            # partitions [64*(head%2), +64) and zeros elsewhere, so a dense
            # 128-lane dot against kT yields exactly that head's scores.
            # The zeroing is folded into the psum eviction via a 0/1
            # per-partition scale.
            qTz = cpool.tile([128, 8, 512], bf16)
            kT = cpool.tile([128, 4, WIN], bf16)
            zm = cpool.tile([128, 2], f32)
            nc.gpsimd.memset(zm[0:64, 0:1], 1.0)
            nc.gpsimd.memset(zm[64:128, 0:1], 0.0)
            nc.gpsimd.memset(zm[0:64, 1:2], 0.0)
            nc.gpsimd.memset(zm[64:128, 1:2], 1.0)

            def emit_proj_q(hdt):
                pq = pbig.tile([128, 2, 512], f32, tag="pbig")
                pqv = pq[:, 0]
                for dc in range(4):
                    nc.tensor.matmul(
                        pqv, wqk_sb[:, dc, 128 * hdt:128 * hdt + 128],
                        hst_sb[:, dc, 7:519], start=(dc == 0), stop=(dc == 3),
                    )
                nc.scalar.activation(qTz[:, 2 * hdt], pqv, AF.Copy,
                                     scale=zm[:, 0:1])
                nc.scalar.activation(qTz[:, 2 * hdt + 1], pqv, AF.Copy,
                                     scale=zm[:, 1:2])

            def emit_proj_k(hdt):
                pk = pbig.tile([128, 2, 512], f32, tag="pbig")
                for half in range(2):
                    pkv = pk[:, half, 0:263]
                    cs = 263 * half
                    for dc in range(4):
                        nc.tensor.matmul(
                            pkv, wqk_sb[:, dc, 512 + 128 * hdt:512 + 128 * hdt + 128],
                            hst_sb[:, dc, cs:cs + 263],
                            start=(dc == 0), stop=(dc == 3),
                        )
                    nc.vector.tensor_copy(kT[:, hdt, cs:cs + 263], pkv)

            # ---- banded attention per row tile, 4-head score groups ----
            def emit_scores(t, g):
                R, W = R_LIST[t], W_LIST[t]
                psc = pscp.tile([128, 456], f32, tag="psc")
                for h4 in range(4):
                    head = 4 * g + h4
                    hdt = head // 2
                    nc.tensor.matmul(
                        psc[0:W, 114 * h4:114 * h4 + R],
                        kT[0:128, hdt, 114 * t:114 * t + W],
                        qTz[0:128, head, 114 * t:114 * t + R],
                        start=True, stop=True,
                    )
                return psc

            def emit_exp(t, g, psc):
                R, W = R_LIST[t], W_LIST[t]
                E = epool.tile([128, 4, 114], bf16, tag="E")
                pv = psc[:].rearrange("p (h r) -> p h r", h=4)
                # fold 1/sqrt(dh) into the exp's scale
                nc.scalar.activation(E[0:W, :, 0:R], pv[0:W, :, 0:R], AF.Exp,
                                     scale=1.0 / (DH ** 0.5))
                # mask (DVE), band rowsum across partitions (Pool), normalize (DVE)
                Em = epool.tile([128, 4, 114], bf16, tag="Em")
                mb = mask_sb[0:W, t, 0:R].unsqueeze(1).broadcast_to((W, 4, R))
                nc.vector.tensor_tensor(out=Em[0:W, :, 0:R], in0=E[0:W, :, 0:R],
                                        in1=mb, op=MUL)
                rs = epool.tile([128, 4, 114], f32, tag="rs")
                nc.gpsimd.partition_all_reduce(
                    rs[0:W, :, 0:R], Em[0:W, :, 0:R], channels=W,
                    reduce_op=bass_isa.ReduceOp.add,
                )
                rsr = epool.tile([128, 4, 114], f32, tag="rsr")
                nc.vector.reciprocal_approx_fast(rsr[0:W, :, 0:R], rs[0:W, :, 0:R])
                En = epool.tile([128, 4, 114], bf16, tag="En")
                nc.vector.tensor_tensor(out=En[0:W, :, 0:R], in0=Em[0:W, :, 0:R],
                                        in1=rsr[0:W, :, 0:R], op=MUL)
                return En

            # eviction engine per head-pair: balance ACT/DVE
            EVICT_ENG = [0, 1, 0, 1]  # 0=ACT 1=DVE

            ctx_state = {}

            def emit_ctx_pair(t, pair, En):
                R, W = R_LIST[t], W_LIST[t]
                if pair == 0:
                    o_t = opool.tile([128, 8, 512], bf16, tag="o")
                    ctx_state[t] = o_t
                o_t = ctx_state[t]
                pc = pbig.tile([128, 2, 512], f32, tag="pbig")
                for j in range(2):
                    head = 2 * pair + j
                    nc.tensor.matmul(
                        pc[0:R, j], En[0:W, head % 4, 0:R], hsw_sb[0:W, t],
                        start=True, stop=True,
                    )
                src = pc[0:R]
                dst = o_t[0:R, 2 * pair:2 * pair + 2]
                if EVICT_ENG[pair] == 0:
                    nc.scalar.activation(dst, src, AF.Copy)
                else:
                    nc.vector.tensor_copy(dst, src)
                if pair == 3:
                    nc.sync.dma_start(out=out_r[114 * t:114 * t + R],
                                      in_=ctx_state.pop(t)[0:R])

            # ---- emission: software-pipeline tiles so the exp/mask/rowsum
            # chain of tile t hides behind the scores matmuls of tiles t+1/t+2
            for hdt in range(4):
                emit_proj_q(hdt)
                emit_proj_k(hdt)

            def emit_tile_scores(t):
                return (emit_exp(t, 0, emit_scores(t, 0)),
                        emit_exp(t, 1, emit_scores(t, 1)))

            Ens = {0: emit_tile_scores(0), 1: emit_tile_scores(1)}
            for t in range(NT):
                if t + 2 < NT:
                    Ens[t + 2] = emit_tile_scores(t + 2)
                En = Ens.pop(t)
                for p in range(4):
                    emit_ctx_pair(t, p, En[0 if p < 2 else 1])

    nc.compile()
    return nc


def _get_nc():
    if "nc" not in _NC_CACHE:
        _NC_CACHE["nc"] = _build_nc()
    return _NC_CACHE["nc"]


def kernel(hidden_states, Wq, Wk):
    global LAST_RESULTS
    from concourse import bass_utils

    B = hidden_states.shape[0]
    hs_bf = np.asarray(hidden_states).astype(BF)
    wq_bf = np.asarray(Wq).astype(BF)
    wk_bf = np.asarray(Wk).astype(BF)

    # [128, 4, 1024] lhsT packing of Wq|Wk
    wqk_host = np.ascontiguousarray(
        np.concatenate(
            [wq_bf.reshape(4, 128, D), wk_bf.reshape(4, 128, D)], axis=2
        ).transpose(1, 0, 2)
    )

    jj = np.arange(128)[:, None]
    ii = np.arange(114)[None, :]

    in_maps = []
    for c in range(NCORES):
        b, quad = c // 4, c % 4
        gs0 = CR * quad
        lo = gs0 - 7
        pad = np.zeros((WINPAD, D), BF)
        s0, s1 = max(lo, 0), min(lo + WINPAD, S)
        pad[s0 - lo:s1 - lo] = hs_bf[b, s0:s1]

        hst_host = np.ascontiguousarray(
            pad[:WIN].T.reshape(4, 128, WIN).transpose(1, 0, 2)
        )
        hsw_host = np.empty((128, 5, 512), BF)
        for t in range(5):
            hsw_host[:, t] = pad[114 * t:114 * t + 128]
        mask_host = np.zeros((128, 5, 114), BF)
        for t in range(5):
            gj = lo + 114 * t + jj
            band = ((jj - ii >= 0) & (jj - ii <= 14)
                    & (gj >= 0) & (gj < S) & (114 * t + ii < CR))
            mask_host[:, t] = band.astype(BF)

        in_maps.append({
            "wqk": wqk_host,
            "hst": hst_host,
            "hsw": np.ascontiguousarray(hsw_host),
            "maskt": np.ascontiguousarray(mask_host),
        })

    nc = _get_nc()
    res = bass_utils.run_bass_kernel_spmd(
        nc, in_maps, core_ids=list(range(NCORES)), trace=TRACE,
    )
    LAST_RESULTS = res

    out = np.empty((B, H, S, D), np.float32)
    for c in range(NCORES):
        b, quad = c // 4, c % 4
        out[b, :, CR * quad:CR * quad + CR, :] = \
            np.asarray(res.results[c]["out"]).astype(np.float32)
    return out
